# revision 25
# baseline (speedup 1.0000x reference)
"""Trainium2 Bass kernel: cached decoder multi-head self-attention (B=16, S=1,
D=2048, H=16, L=4096), tensor-parallel over heads across 8 NeuronCores.

Per core (2 heads x 16 batches = 32 (b,h) pairs):
  - QKV projections for its head group (column-sharded weights),
  - streaming attention over the 4096-entry KV cache (+1 new token),
  - row-sharded output projection producing a partial (16, 2048) result.
Host packs per-pair SBUF-image cache buffers (K transposed to (dk, L); V
chunk-partition-major) so device DMA is fully contiguous, and sums the 8
partial outputs. Softmax normalization is deferred past the PV matmul
(single reciprocal scale per pair).
"""
import math
import os
import re
import sys

sys.path.insert(0, "/opt/trn_rl_repo")

import numpy as np

import concourse.bass as bass
import concourse.tile as tile
from concourse import bacc, mybir
from concourse import bass_utils

# ---------------------------------------------------------------- problem dims
B, S, D, H, L, DK = 16, 1, 2048, 16, 4096, 128
NCORES = 8
HLOC = H // NCORES            # heads per core = 2
NPAIR = B * HLOC              # (b, h) pairs per core = 32
NCH = L // 128                # 128-key chunks per pair = 32
DCH = D // 128                # 128-row chunks of the model dim = 16
SCALE = 1.0 / math.sqrt(DK)
F32 = mybir.dt.float32
F32R = mybir.dt.float32r
USE_F32R = True               # reduced-precision PE mode for the cache matmuls


def build_nc():
    nc = bacc.Bacc("TRN2", target_bir_lowering=False, debug=False, num_devices=1)

    dt = F32
    dtr = F32R if USE_F32R else F32
    kt_img = nc.dram_tensor("kt_img", [NPAIR, 128, L], dtr, kind="ExternalInput").ap()
    v_img = nc.dram_tensor("v_img", [NPAIR, 128, L], dtr, kind="ExternalInput").ap()
    wq_d = nc.dram_tensor("wq", [D, HLOC * DK], dt, kind="ExternalInput").ap()
    wk_d = nc.dram_tensor("wk", [D, HLOC * DK], dt, kind="ExternalInput").ap()
    wv_d = nc.dram_tensor("wv", [D, HLOC * DK], dt, kind="ExternalInput").ap()
    wo_d = nc.dram_tensor("wo", [HLOC * DK, D], dt, kind="ExternalInput").ap()
    bq_d = nc.dram_tensor("bq", [1, HLOC * DK], dt, kind="ExternalInput").ap()
    bk_d = nc.dram_tensor("bk", [1, HLOC * DK], dt, kind="ExternalInput").ap()
    bv_d = nc.dram_tensor("bv", [1, HLOC * DK], dt, kind="ExternalInput").ap()
    xT_d = nc.dram_tensor("xT", [D, B], dt, kind="ExternalInput").ap()
    eye_d = nc.dram_tensor("eye16", [B, B], dt, kind="ExternalInput").ap()
    onp_d = nc.dram_tensor("ones_p", [128, 1], dt, kind="ExternalInput").ap()
    onr_d = nc.dram_tensor("ones_r", [1, 128], dt, kind="ExternalInput").ap()
    onb_d = nc.dram_tensor("ones_b", [1, B], dt, kind="ExternalInput").ap()

    out_d = nc.dram_tensor("out_p", [B, D], dt, kind="ExternalOutput").ap()
    kn_d = nc.dram_tensor("k_new", [128, NPAIR], dt, kind="ExternalOutput").ap()
    vn_d = nc.dram_tensor("v_new", [B, HLOC * DK], dt, kind="ExternalOutput").ap()
    DBG = os.environ.get("KDBG", "0") == "1"
    if DBG:
        dbg_q = nc.dram_tensor("dbg_q", [128, 2 * NPAIR], dt, kind="ExternalOutput").ap()
        dbg_e = nc.dram_tensor("dbg_e", [B, HLOC], dt, kind="ExternalOutput").ap()
        dbg_d = nc.dram_tensor("dbg_d", [1, NPAIR], dt, kind="ExternalOutput").ap()
        dbg_r = nc.dram_tensor("dbg_r", [128, NPAIR], dt, kind="ExternalOutput").ap()
        dbg_a = nc.dram_tensor("dbg_a", [128, NPAIR], dt, kind="ExternalOutput").ap()
        dbg_au = nc.dram_tensor("dbg_au", [128, 2 * NPAIR], dt, kind="ExternalOutput").ap()

    Exp = mybir.ActivationFunctionType.Exp

    with tile.TileContext(nc) as tc:
        with (
            tc.tile_pool(name="const", bufs=1) as cpool,
            tc.tile_pool(name="wts", bufs=1) as wpool,
            tc.tile_pool(name="proj", bufs=1) as ppool,
            tc.tile_pool(name="kt", bufs=2) as ktpool,
            tc.tile_pool(name="vv", bufs=2) as vvpool,
            tc.tile_pool(name="probs", bufs=3) as prpool,
            tc.tile_pool(name="fin", bufs=1) as fpool,
            tc.tile_pool(name="psA", bufs=1, space="PSUM") as psA,
            tc.tile_pool(name="psD", bufs=1, space="PSUM") as psD,
        ):
            # ---------------- constants / weights / activations to SBUF
            eye_sb = cpool.tile([B, B], dt)
            nc.sync.dma_start(eye_sb[:], eye_d[:])
            onp_sb = cpool.tile([128, 1], dt)
            nc.sync.dma_start(onp_sb[:], onp_d[:])
            onr_sb = cpool.tile([1, 128], dt)
            nc.sync.dma_start(onr_sb[:], onr_d[:])
            onb_sb = cpool.tile([1, B], dt)
            nc.sync.dma_start(onb_sb[:], onb_d[:])
            bq_sb = cpool.tile([1, HLOC * DK], dt)
            nc.sync.dma_start(bq_sb[:], bq_d[:])
            bk_sb = cpool.tile([1, HLOC * DK], dt)
            nc.sync.dma_start(bk_sb[:], bk_d[:])
            bv_sb = cpool.tile([1, HLOC * DK], dt)
            nc.sync.dma_start(bv_sb[:], bv_d[:])
            xT_sb = cpool.tile([128, DCH * B], dt)
            nc.sync.dma_start(
                xT_sb[:].rearrange("p (c b) -> p c b", c=DCH),
                xT_d.rearrange("(c p) b -> p c b", p=128),
            )
            wq_sb = wpool.tile([128, DCH * HLOC * DK], dt)
            nc.sync.dma_start(
                wq_sb[:].rearrange("p (c n) -> p c n", c=DCH),
                wq_d.rearrange("(c p) n -> p c n", p=128),
            )
            wk_sb = wpool.tile([128, DCH * HLOC * DK], dt)
            nc.sync.dma_start(
                wk_sb[:].rearrange("p (c n) -> p c n", c=DCH),
                wk_d.rearrange("(c p) n -> p c n", p=128),
            )
            wv_sb = wpool.tile([128, DCH * HLOC * DK], dt)
            nc.sync.dma_start(
                wv_sb[:].rearrange("p (c n) -> p c n", c=DCH),
                wv_d.rearrange("(c p) n -> p c n", p=128),
            )
            wo_sb = wpool.tile([128, HLOC * D], dt)
            nc.sync.dma_start(
                wo_sb[:].rearrange("p (c n) -> p c n", c=HLOC),
                wo_d.rearrange("(c p) n -> p c n", p=128),
            )

            HD = HLOC * DK  # 256

            # ---------------- projections
            # qT2: each pair's q duplicated into two adjacent columns (fp32r
            # matmuls need even moving/dst free counts). col = 2*pair + {0,1}
            qT2_sb = ppool.tile([128, 2 * NPAIR], dtr)
            kT_sb = ppool.tile([128, NPAIR], dt)
            v_sb = ppool.tile([B, HD], dt)         # (batch, hl*dk)
            e_sb = ppool.tile([B, HLOC], dt)       # exp(new-key scores)

            with tc.tile_pool(name="psP", bufs=1, space="PSUM") as psP:
                for wsb, bsb, which in ((wq_sb, bq_sb, "q"), (wk_sb, bk_sb, "k")):
                    for hl in range(HLOC):
                        ps_qt = psP.tile([128, B], dt, tag="qt")
                        for c in range(DCH):
                            nc.tensor.matmul(
                                ps_qt[:],
                                wsb[:, c * HD + hl * DK : c * HD + (hl + 1) * DK],
                                xT_sb[:, c * B : (c + 1) * B],
                                start=(c == 0),
                                stop=False,
                            )
                        nc.tensor.matmul(
                            ps_qt[:],
                            bsb[0:1, hl * DK : (hl + 1) * DK],
                            onb_sb[:],
                            start=False,
                            stop=True,
                        )
                        if which == "q":
                            qslice = qT2_sb[
                                :, 2 * hl * B : 2 * (hl + 1) * B
                            ].rearrange("p (c t) -> p c t", t=2)
                            nc.vector.tensor_copy(
                                qslice[:, :, 0:1],
                                ps_qt[:].rearrange("p (c t) -> p c t", t=1),
                            )
                            nc.vector.tensor_copy(
                                qslice[:, :, 1:2],
                                ps_qt[:].rearrange("p (c t) -> p c t", t=1),
                            )
                        else:
                            nc.vector.tensor_copy(
                                kT_sb[:, hl * B : (hl + 1) * B], ps_qt[:]
                            )
                # V projection, batch-major
                ps_v = psP.tile([B, HD], dt, tag="v")
                for c in range(DCH):
                    nc.tensor.matmul(
                        ps_v[:],
                        xT_sb[:, c * B : (c + 1) * B],
                        wv_sb[:, c * HD : (c + 1) * HD],
                        start=(c == 0),
                        stop=False,
                    )
                nc.tensor.matmul(ps_v[:], onb_sb[:], bv_sb[:], start=False, stop=True)
                nc.vector.tensor_copy(v_sb[:], ps_v[:])

                # new-key scores s_new[b, hl] = q . k_new
                ps_sn = psP.tile([B, HLOC], dt, tag="sn")
                for hl in range(HLOC):
                    tmp = ppool.tile([128, B], dt, tag="tmp")
                    qf32 = (
                        qT2_sb[:, 2 * hl * B : 2 * (hl + 1) * B]
                        .bitcast(F32)
                        .rearrange("p (c t) -> p c t", t=2)
                    )
                    nc.vector.tensor_mul(
                        tmp[:].rearrange("p (c t) -> p c t", t=1),
                        qf32[:, :, 0:1],
                        kT_sb[:, hl * B : (hl + 1) * B].rearrange(
                            "p (c t) -> p c t", t=1
                        ),
                    )
                    nc.tensor.matmul(
                        ps_sn[:, hl : hl + 1], tmp[:], onp_sb[:],
                        start=True, stop=True,
                    )
                nc.scalar.activation(e_sb[:], ps_sn[:], Exp, scale=SCALE)

            # ---------------- pair loop: stream the KV cache
            # fp32r duplicated-column layout: scores/attn psums hold each
            # value twice (cols 2i, 2i+1); part sums come out doubled and the
            # 2.0-valued broadcast constant cancels the factor at normalize.
            ps_attn = psA.tile([128, 2 * NPAIR], dt)
            ps_d = psD.tile([1, NPAIR], dt)        # doubled denominators

            with tc.tile_pool(name="psS", bufs=3, space="PSUM") as psS:
                for p in range(NPAIR):
                    kt_t = ktpool.tile([128, L], dtr, tag="kt")
                    nc.sync.dma_start(kt_t[:], kt_img[p, :, :])
                    v_t = vvpool.tile([128, L], dtr, tag="vt")
                    nc.sync.dma_start(v_t[:], v_img[p, :, :])

                    ps_s = psS.tile([128, 2 * NCH], dt, tag="s")
                    for j in range(NCH):
                        nc.tensor.matmul(
                            ps_s[:, 2 * j : 2 * j + 2],
                            kt_t[:, j * 128 : (j + 1) * 128],
                            qT2_sb[:, 2 * p : 2 * p + 2],
                            start=True,
                            stop=True,
                        )
                    probs_t = prpool.tile([128, 2 * NCH], dtr, tag="probs")
                    part_t = prpool.tile([128, 1], dt, tag="part")
                    nc.scalar.activation(
                        probs_t[:], ps_s[:], Exp, scale=SCALE, accum_out=part_t[:]
                    )
                    # doubled denominator partial (sums both duplicate cols)
                    nc.tensor.matmul(
                        ps_d[0:1, p : p + 1], part_t[:], onp_sb[:],
                        start=True, stop=True,
                    )
                    for j in range(NCH):
                        nc.tensor.matmul(
                            ps_attn[:, 2 * p : 2 * p + 2],
                            v_t[:, j * 128 : (j + 1) * 128],
                            probs_t[:, 2 * j : 2 * j + 2],
                            start=(j == 0),
                            stop=(j == NCH - 1),
                        )

            # ---------------- new-token contributions + normalization
            with (
                tc.tile_pool(name="psF", bufs=1, space="PSUM") as psF,
                tc.tile_pool(name="psO", bufs=2, space="PSUM") as psO,
            ):
                # new-key contributions in their own (locally-closed) psums
                ps_nk = psF.tile([128, NPAIR], dt, tag="nk")
                ps_de = psF.tile([1, NPAIR], dt, tag="de")
                for hl in range(HLOC):
                    diag_t = fpool.tile([B, B], dt, tag="diag")
                    nc.vector.tensor_scalar_mul(
                        diag_t[:], eye_sb[:], e_sb[:, hl : hl + 1]
                    )
                    nc.tensor.matmul(
                        ps_nk[:, hl * B : (hl + 1) * B],
                        v_sb[:, hl * DK : (hl + 1) * DK],
                        diag_t[:],
                        start=True,
                        stop=True,
                    )
                    nc.tensor.matmul(
                        ps_de[0:1, hl * B : (hl + 1) * B],
                        e_sb[:, hl : hl + 1],
                        eye_sb[:],
                        start=True,
                        stop=True,
                    )
                # d_total = 0.5 * (doubled cached sums) + e_new
                d_sb = fpool.tile([1, NPAIR], dt)
                nc.vector.tensor_copy(d_sb[:], ps_d[:])
                dt_sb = fpool.tile([1, NPAIR], dt)
                nc.vector.scalar_tensor_tensor(
                    dt_sb[:], d_sb[:], 0.5, ps_de[:],
                    op0=mybir.AluOpType.mult, op1=mybir.AluOpType.add,
                )
                r_sb = fpool.tile([1, NPAIR], dt)
                nc.vector.reciprocal(r_sb[:], dt_sb[:])
                ps_r = psF.tile([128, NPAIR], dt, tag="r")
                nc.tensor.matmul(ps_r[:], onr_sb[:], r_sb[:], start=True, stop=True)
                rbc_sb = fpool.tile([128, NPAIR], dt)
                nc.vector.tensor_copy(rbc_sb[:], ps_r[:])
                # attn = (cached-attn + new-key-attn) / d
                nk_sb = fpool.tile([128, NPAIR], dt)
                nc.vector.tensor_copy(nk_sb[:], ps_nk[:])
                asum_sb = fpool.tile([128, NPAIR], dt)
                nc.vector.tensor_add(
                    asum_sb[:].rearrange("p (c t) -> p c t", t=1),
                    ps_attn[:].rearrange("p (c t) -> p c t", t=2)[:, :, 0:1],
                    nk_sb[:].rearrange("p (c t) -> p c t", t=1),
                )
                attn_sb = fpool.tile([128, NPAIR], dt)
                nc.vector.tensor_mul(attn_sb[:], asum_sb[:], rbc_sb[:])

                # output projection: out_partial = attn @ Wo_shard
                out_sb = fpool.tile([B, D], dt)
                for n in range(4):
                    ps_o = psO.tile([B, 512], dt, tag="o")
                    for hl in range(HLOC):
                        nc.tensor.matmul(
                            ps_o[:],
                            attn_sb[:, hl * B : (hl + 1) * B],
                            wo_sb[:, hl * D + n * 512 : hl * D + (n + 1) * 512],
                            start=(hl == 0),
                            stop=(hl == HLOC - 1),
                        )
                    nc.vector.tensor_copy(out_sb[:, n * 512 : (n + 1) * 512], ps_o[:])

                nc.sync.dma_start(out_d[:], out_sb[:])
                nc.sync.dma_start(kn_d[:], kT_sb[:])
                nc.sync.dma_start(vn_d[:], v_sb[:])
                if DBG:
                    dq = fpool.tile([128, 2 * NPAIR], dt, tag="dq")
                    nc.vector.tensor_copy(dq[:], qT2_sb[:].bitcast(F32))
                    nc.sync.dma_start(dbg_q[:], dq[:])
                    nc.sync.dma_start(dbg_e[:], e_sb[:])
                    nc.sync.dma_start(dbg_d[:], d_sb[:])
                    nc.sync.dma_start(dbg_r[:], rbc_sb[:])
                    nc.sync.dma_start(dbg_a[:], attn_sb[:])
                    dau = fpool.tile([128, 2 * NPAIR], dt, tag="dau")
                    nc.vector.tensor_copy(dau[:], ps_attn[:])
                    nc.sync.dma_start(dbg_au[:], dau[:])

    nc.compile()
    return nc


_NC = None


def _get_nc():
    global _NC
    if _NC is None:
        _NC = build_nc()
    return _NC


def build_in_maps(inputs):
    x = np.ascontiguousarray(np.asarray(inputs["x"], dtype=np.float32)).reshape(B, D)
    cache_k = np.asarray(inputs["cache_k"], dtype=np.float32)
    cache_v = np.asarray(inputs["cache_v"], dtype=np.float32)
    Wq = np.asarray(inputs["Wq"], dtype=np.float32)
    Wk = np.asarray(inputs["Wk"], dtype=np.float32)
    Wv = np.asarray(inputs["Wv"], dtype=np.float32)
    Wo = np.asarray(inputs["Wo"], dtype=np.float32)
    bq = np.asarray(inputs["bq"], dtype=np.float32)
    bk = np.asarray(inputs["bk"], dtype=np.float32)
    bv = np.asarray(inputs["bv"], dtype=np.float32)
    assert int(inputs.get("num_heads", H)) == H

    xT = np.ascontiguousarray(x.T)                       # (D, B)
    eye = np.eye(B, dtype=np.float32)
    ones_p = np.ones((128, 1), np.float32)
    ones_r = np.ones((1, 128), np.float32)
    ones_b = np.ones((1, B), np.float32)

    in_maps = []
    for c in range(NCORES):
        h0 = c * HLOC
        sl = slice(h0 * DK, (h0 + HLOC) * DK)
        # K cache, transposed per pair to (dk, L); pair index = hl*16 + b
        ck = cache_k[:, h0 : h0 + HLOC]                  # (B, HLOC, L, DK)
        kt = np.ascontiguousarray(
            ck.transpose(1, 0, 3, 2).reshape(NPAIR, 128, L)
        )
        # V cache, chunk-partition-major: img[pair, p, j*128+d] = V[j*128+p, d]
        cv = cache_v[:, h0 : h0 + HLOC].reshape(B, HLOC, NCH, 128, DK)
        vi = np.ascontiguousarray(
            cv.transpose(1, 0, 3, 2, 4).reshape(NPAIR, 128, L)
        )
        in_maps.append(
            {
                "kt_img": kt,
                "v_img": vi,
                "wq": np.ascontiguousarray(Wq[:, sl]),
                "wk": np.ascontiguousarray(Wk[:, sl]),
                "wv": np.ascontiguousarray(Wv[:, sl]),
                "wo": np.ascontiguousarray(Wo[sl, :]),
                "bq": np.ascontiguousarray(bq[sl]).reshape(1, -1),
                "bk": np.ascontiguousarray(bk[sl]).reshape(1, -1),
                "bv": np.ascontiguousarray(bv[sl]).reshape(1, -1),
                "xT": xT,
                "eye16": eye,
                "ones_p": ones_p,
                "ones_r": ones_r,
                "ones_b": ones_b,
            }
        )
    return in_maps


def kernel(**inputs):
    cache_k = np.asarray(inputs["cache_k"], dtype=np.float32)
    cache_v = np.asarray(inputs["cache_v"], dtype=np.float32)
    bo = np.asarray(inputs["bo"], dtype=np.float32)

    in_maps = build_in_maps(inputs)
    nc = _get_nc()
    res = bass_utils.run_bass_kernel_spmd(nc, in_maps, core_ids=list(range(NCORES)))

    out = np.zeros((B, D), np.float64)
    k_new = np.empty((B, H, 1, DK), np.float32)
    v_new = np.empty((B, H, 1, DK), np.float32)
    for c in range(NCORES):
        h0 = c * HLOC
        out += res.results[c]["out_p"]
        kT = res.results[c]["k_new"]                     # (128, NPAIR)
        vn = res.results[c]["v_new"]                     # (B, HLOC*DK)
        for hl in range(HLOC):
            k_new[:, h0 + hl, 0, :] = kT[:, hl * B : (hl + 1) * B].T
            v_new[:, h0 + hl, 0, :] = vn[:, hl * DK : (hl + 1) * DK]

    out = (out + bo[None, :]).astype(np.float32).reshape(B, S, D)
    K_full = np.concatenate([cache_k, k_new], axis=2)
    V_full = np.concatenate([cache_v, v_new], axis=2)
    return out, K_full, V_full


# revision 28
# speedup vs baseline: 1.1506x; 1.1506x over previous
"""Trainium2 Bass kernel: cached decoder multi-head self-attention (B=16, S=1,
D=2048, H=16, L=4096), tensor-parallel over heads across 8 NeuronCores.

Per core (2 heads x 16 batches = 32 (b,h) pairs):
  - QKV projections for its head group (column-sharded weights),
  - streaming attention over the 4096-entry KV cache (+1 new token),
  - row-sharded output projection producing a partial (16, 2048) result.
Host packs per-pair SBUF-image cache buffers (K transposed to (dk, L); V
chunk-partition-major) so device DMA is fully contiguous, and sums the 8
partial outputs. Softmax normalization is deferred past the PV matmul
(single reciprocal scale per pair).
"""
import math
import os
import re
import sys

sys.path.insert(0, "/opt/trn_rl_repo")

import numpy as np

import concourse.bass as bass
import concourse.tile as tile
from concourse import bacc, mybir
from concourse import bass_utils

# ---------------------------------------------------------------- problem dims
B, S, D, H, L, DK = 16, 1, 2048, 16, 4096, 128
NCORES = 8
HLOC = H // NCORES            # heads per core = 2
NPAIR = B * HLOC              # (b, h) pairs per core = 32
NCH = L // 128                # 128-key chunks per pair = 32
DCH = D // 128                # 128-row chunks of the model dim = 16
SCALE = 1.0 / math.sqrt(DK)
F32 = mybir.dt.float32
F32R = mybir.dt.float32r
USE_F32R = True               # reduced-precision PE mode for the cache matmuls


def build_nc():
    nc = bacc.Bacc("TRN2", target_bir_lowering=False, debug=False, num_devices=1)

    dt = F32
    dtr = F32R if USE_F32R else F32
    kt_img = nc.dram_tensor("kt_img", [NPAIR, 128, L], dtr, kind="ExternalInput").ap()
    v_img = nc.dram_tensor("v_img", [NPAIR, 128, L], dt, kind="ExternalInput").ap()
    wq_d = nc.dram_tensor("wq", [D, HLOC * DK], dt, kind="ExternalInput").ap()
    wk_d = nc.dram_tensor("wk", [D, HLOC * DK], dt, kind="ExternalInput").ap()
    wv_d = nc.dram_tensor("wv", [D, HLOC * DK], dt, kind="ExternalInput").ap()
    wo_d = nc.dram_tensor("wo", [HLOC * DK, D], dt, kind="ExternalInput").ap()
    bq_d = nc.dram_tensor("bq", [1, HLOC * DK], dt, kind="ExternalInput").ap()
    bk_d = nc.dram_tensor("bk", [1, HLOC * DK], dt, kind="ExternalInput").ap()
    bv_d = nc.dram_tensor("bv", [1, HLOC * DK], dt, kind="ExternalInput").ap()
    xT_d = nc.dram_tensor("xT", [D, B], dt, kind="ExternalInput").ap()
    eye_d = nc.dram_tensor("eye16", [B, B], dt, kind="ExternalInput").ap()
    onp_d = nc.dram_tensor("ones_p", [128, 1], dt, kind="ExternalInput").ap()
    onr_d = nc.dram_tensor("ones_r", [1, 128], dt, kind="ExternalInput").ap()
    onb_d = nc.dram_tensor("ones_b", [1, B], dt, kind="ExternalInput").ap()

    out_d = nc.dram_tensor("out_p", [B, D], dt, kind="ExternalOutput").ap()
    kn_d = nc.dram_tensor("k_new", [128, NPAIR], dt, kind="ExternalOutput").ap()
    vn_d = nc.dram_tensor("v_new", [B, HLOC * DK], dt, kind="ExternalOutput").ap()
    DBG = os.environ.get("KDBG", "0") == "1"
    if DBG:
        dbg_q = nc.dram_tensor("dbg_q", [128, 2 * NPAIR], dt, kind="ExternalOutput").ap()
        dbg_e = nc.dram_tensor("dbg_e", [B, HLOC], dt, kind="ExternalOutput").ap()
        dbg_d = nc.dram_tensor("dbg_d", [1, NPAIR], dt, kind="ExternalOutput").ap()
        dbg_r = nc.dram_tensor("dbg_r", [128, NPAIR], dt, kind="ExternalOutput").ap()
        dbg_a = nc.dram_tensor("dbg_a", [128, NPAIR], dt, kind="ExternalOutput").ap()
        dbg_au = nc.dram_tensor("dbg_au", [128, 2 * NPAIR], dt, kind="ExternalOutput").ap()

    Exp = mybir.ActivationFunctionType.Exp

    with tile.TileContext(nc) as tc:
        with (
            tc.tile_pool(name="const", bufs=1) as cpool,
            tc.tile_pool(name="wts", bufs=1) as wpool,
            tc.tile_pool(name="proj", bufs=1) as ppool,
            tc.tile_pool(name="kt", bufs=2) as ktpool,
            tc.tile_pool(name="vv", bufs=2) as vvpool,
            tc.tile_pool(name="probs", bufs=3) as prpool,
            tc.tile_pool(name="fin", bufs=1) as fpool,
            tc.tile_pool(name="psA", bufs=1, space="PSUM") as psA,
            tc.tile_pool(name="psD", bufs=1, space="PSUM") as psD,
        ):
            # ---------------- constants / weights / activations to SBUF
            eye_sb = cpool.tile([B, B], dt)
            nc.sync.dma_start(eye_sb[:], eye_d[:])
            onp_sb = cpool.tile([128, 1], dt)
            nc.sync.dma_start(onp_sb[:], onp_d[:])
            onr_sb = cpool.tile([1, 128], dt)
            nc.sync.dma_start(onr_sb[:], onr_d[:])
            onb_sb = cpool.tile([1, B], dt)
            nc.sync.dma_start(onb_sb[:], onb_d[:])
            bq_sb = cpool.tile([1, HLOC * DK], dt)
            nc.sync.dma_start(bq_sb[:], bq_d[:])
            bk_sb = cpool.tile([1, HLOC * DK], dt)
            nc.sync.dma_start(bk_sb[:], bk_d[:])
            bv_sb = cpool.tile([1, HLOC * DK], dt)
            nc.sync.dma_start(bv_sb[:], bv_d[:])
            xT_sb = cpool.tile([128, DCH * B], dt)
            nc.sync.dma_start(
                xT_sb[:].rearrange("p (c b) -> p c b", c=DCH),
                xT_d.rearrange("(c p) b -> p c b", p=128),
            )
            wq_sb = wpool.tile([128, DCH * HLOC * DK], dt)
            nc.sync.dma_start(
                wq_sb[:].rearrange("p (c n) -> p c n", c=DCH),
                wq_d.rearrange("(c p) n -> p c n", p=128),
            )
            wk_sb = wpool.tile([128, DCH * HLOC * DK], dt)
            nc.sync.dma_start(
                wk_sb[:].rearrange("p (c n) -> p c n", c=DCH),
                wk_d.rearrange("(c p) n -> p c n", p=128),
            )
            wv_sb = wpool.tile([128, DCH * HLOC * DK], dt)
            nc.sync.dma_start(
                wv_sb[:].rearrange("p (c n) -> p c n", c=DCH),
                wv_d.rearrange("(c p) n -> p c n", p=128),
            )
            wo_sb = wpool.tile([128, HLOC * D], dt)
            nc.sync.dma_start(
                wo_sb[:].rearrange("p (c n) -> p c n", c=HLOC),
                wo_d.rearrange("(c p) n -> p c n", p=128),
            )

            HD = HLOC * DK  # 256

            # ---------------- projections
            # qT2: each pair's q duplicated into two adjacent columns (fp32r
            # matmuls need even moving/dst free counts). col = 2*pair + {0,1}
            qT2_sb = ppool.tile([128, 2 * NPAIR], dtr)
            kT_sb = ppool.tile([128, NPAIR], dt)
            v_sb = ppool.tile([B, HD], dt)         # (batch, hl*dk)
            e_sb = ppool.tile([B, HLOC], dt)       # exp(new-key scores)

            with tc.tile_pool(name="psP", bufs=1, space="PSUM") as psP:
                for wsb, bsb, which in ((wq_sb, bq_sb, "q"), (wk_sb, bk_sb, "k")):
                    for hl in range(HLOC):
                        ps_qt = psP.tile([128, B], dt, tag="qt")
                        for c in range(DCH):
                            nc.tensor.matmul(
                                ps_qt[:],
                                wsb[:, c * HD + hl * DK : c * HD + (hl + 1) * DK],
                                xT_sb[:, c * B : (c + 1) * B],
                                start=(c == 0),
                                stop=False,
                            )
                        nc.tensor.matmul(
                            ps_qt[:],
                            bsb[0:1, hl * DK : (hl + 1) * DK],
                            onb_sb[:],
                            start=False,
                            stop=True,
                        )
                        if which == "q":
                            qslice = qT2_sb[
                                :, 2 * hl * B : 2 * (hl + 1) * B
                            ].rearrange("p (c t) -> p c t", t=2)
                            nc.vector.tensor_copy(
                                qslice[:, :, 0:1],
                                ps_qt[:].rearrange("p (c t) -> p c t", t=1),
                            )
                            nc.vector.tensor_copy(
                                qslice[:, :, 1:2],
                                ps_qt[:].rearrange("p (c t) -> p c t", t=1),
                            )
                        else:
                            nc.vector.tensor_copy(
                                kT_sb[:, hl * B : (hl + 1) * B], ps_qt[:]
                            )
                # V projection, batch-major
                ps_v = psP.tile([B, HD], dt, tag="v")
                for c in range(DCH):
                    nc.tensor.matmul(
                        ps_v[:],
                        xT_sb[:, c * B : (c + 1) * B],
                        wv_sb[:, c * HD : (c + 1) * HD],
                        start=(c == 0),
                        stop=False,
                    )
                nc.tensor.matmul(ps_v[:], onb_sb[:], bv_sb[:], start=False, stop=True)
                nc.vector.tensor_copy(v_sb[:], ps_v[:])

                # new-key scores s_new[b, hl] = q . k_new
                ps_sn = psP.tile([B, HLOC], dt, tag="sn")
                for hl in range(HLOC):
                    tmp = ppool.tile([128, B], dt, tag="tmp")
                    qf32 = (
                        qT2_sb[:, 2 * hl * B : 2 * (hl + 1) * B]
                        .bitcast(F32)
                        .rearrange("p (c t) -> p c t", t=2)
                    )
                    nc.vector.tensor_mul(
                        tmp[:].rearrange("p (c t) -> p c t", t=1),
                        qf32[:, :, 0:1],
                        kT_sb[:, hl * B : (hl + 1) * B].rearrange(
                            "p (c t) -> p c t", t=1
                        ),
                    )
                    nc.tensor.matmul(
                        ps_sn[:, hl : hl + 1], tmp[:], onp_sb[:],
                        start=True, stop=True,
                    )
                nc.scalar.activation(e_sb[:], ps_sn[:], Exp, scale=SCALE)

            # ---------------- pair loop: stream the KV cache
            # fp32r duplicated-column layout: scores/attn psums hold each
            # value twice (cols 2i, 2i+1); part sums come out doubled and the
            # 2.0-valued broadcast constant cancels the factor at normalize.
            ps_attn = psA.tile([128, 2 * NPAIR], dt)
            ps_d = psD.tile([1, NPAIR], dt)        # doubled denominators

            with tc.tile_pool(name="psS", bufs=3, space="PSUM") as psS:
                BF16 = mybir.dt.bfloat16
                for p in range(NPAIR):
                    kt_t = ktpool.tile([128, L], dtr, tag="kt")
                    nc.sync.dma_start(kt_t[:], kt_img[p, :, :])
                    # V cast to bf16 in the DMA (SWDGE): halves SBUF traffic
                    # and enables fast-weight-load bf16 PV matmuls
                    v_t = vvpool.tile([128, L], BF16, tag="vt")
                    nc.gpsimd.dma_start(v_t[:], v_img[p, :, :])

                    ps_s = psS.tile([128, 2 * NCH], dt, tag="s")
                    for j in range(NCH):
                        nc.tensor.matmul(
                            ps_s[:, 2 * j : 2 * j + 2],
                            kt_t[:, j * 128 : (j + 1) * 128],
                            qT2_sb[:, 2 * p : 2 * p + 2],
                            start=True,
                            stop=True,
                        )
                    probs_t = prpool.tile([128, 2 * NCH], BF16, tag="probs")
                    part_t = prpool.tile([128, 1], dt, tag="part")
                    nc.scalar.activation(
                        probs_t[:], ps_s[:], Exp, scale=SCALE, accum_out=part_t[:]
                    )
                    # doubled denominator partial (sums both duplicate cols)
                    nc.tensor.matmul(
                        ps_d[0:1, p : p + 1], part_t[:], onp_sb[:],
                        start=True, stop=True,
                    )
                    for j in range(NCH):
                        nc.tensor.matmul(
                            ps_attn[:, 2 * p : 2 * p + 2],
                            v_t[:, j * 128 : (j + 1) * 128],
                            probs_t[:, 2 * j : 2 * j + 2],
                            start=(j == 0),
                            stop=(j == NCH - 1),
                        )

            # ---------------- new-token contributions + normalization
            with (
                tc.tile_pool(name="psF", bufs=1, space="PSUM") as psF,
                tc.tile_pool(name="psO", bufs=2, space="PSUM") as psO,
            ):
                # new-key contributions in their own (locally-closed) psums
                ps_nk = psF.tile([128, NPAIR], dt, tag="nk")
                ps_de = psF.tile([1, NPAIR], dt, tag="de")
                for hl in range(HLOC):
                    diag_t = fpool.tile([B, B], dt, tag="diag")
                    nc.vector.tensor_scalar_mul(
                        diag_t[:], eye_sb[:], e_sb[:, hl : hl + 1]
                    )
                    nc.tensor.matmul(
                        ps_nk[:, hl * B : (hl + 1) * B],
                        v_sb[:, hl * DK : (hl + 1) * DK],
                        diag_t[:],
                        start=True,
                        stop=True,
                    )
                    nc.tensor.matmul(
                        ps_de[0:1, hl * B : (hl + 1) * B],
                        e_sb[:, hl : hl + 1],
                        eye_sb[:],
                        start=True,
                        stop=True,
                    )
                # d_total = 0.5 * (doubled cached sums) + e_new
                d_sb = fpool.tile([1, NPAIR], dt)
                nc.vector.tensor_copy(d_sb[:], ps_d[:])
                dt_sb = fpool.tile([1, NPAIR], dt)
                nc.vector.scalar_tensor_tensor(
                    dt_sb[:], d_sb[:], 0.5, ps_de[:],
                    op0=mybir.AluOpType.mult, op1=mybir.AluOpType.add,
                )
                r_sb = fpool.tile([1, NPAIR], dt)
                nc.vector.reciprocal(r_sb[:], dt_sb[:])
                ps_r = psF.tile([128, NPAIR], dt, tag="r")
                nc.tensor.matmul(ps_r[:], onr_sb[:], r_sb[:], start=True, stop=True)
                rbc_sb = fpool.tile([128, NPAIR], dt)
                nc.vector.tensor_copy(rbc_sb[:], ps_r[:])
                # attn = (cached-attn + new-key-attn) / d
                nk_sb = fpool.tile([128, NPAIR], dt)
                nc.vector.tensor_copy(nk_sb[:], ps_nk[:])
                asum_sb = fpool.tile([128, NPAIR], dt)
                nc.vector.tensor_add(
                    asum_sb[:].rearrange("p (c t) -> p c t", t=1),
                    ps_attn[:].rearrange("p (c t) -> p c t", t=2)[:, :, 0:1],
                    nk_sb[:].rearrange("p (c t) -> p c t", t=1),
                )
                attn_sb = fpool.tile([128, NPAIR], dt)
                nc.vector.tensor_mul(attn_sb[:], asum_sb[:], rbc_sb[:])

                # output projection: out_partial = attn @ Wo_shard
                out_sb = fpool.tile([B, D], dt)
                for n in range(4):
                    ps_o = psO.tile([B, 512], dt, tag="o")
                    for hl in range(HLOC):
                        nc.tensor.matmul(
                            ps_o[:],
                            attn_sb[:, hl * B : (hl + 1) * B],
                            wo_sb[:, hl * D + n * 512 : hl * D + (n + 1) * 512],
                            start=(hl == 0),
                            stop=(hl == HLOC - 1),
                        )
                    nc.vector.tensor_copy(out_sb[:, n * 512 : (n + 1) * 512], ps_o[:])

                nc.sync.dma_start(out_d[:], out_sb[:])
                nc.sync.dma_start(kn_d[:], kT_sb[:])
                nc.sync.dma_start(vn_d[:], v_sb[:])
                if DBG:
                    dq = fpool.tile([128, 2 * NPAIR], dt, tag="dq")
                    nc.vector.tensor_copy(dq[:], qT2_sb[:].bitcast(F32))
                    nc.sync.dma_start(dbg_q[:], dq[:])
                    nc.sync.dma_start(dbg_e[:], e_sb[:])
                    nc.sync.dma_start(dbg_d[:], d_sb[:])
                    nc.sync.dma_start(dbg_r[:], rbc_sb[:])
                    nc.sync.dma_start(dbg_a[:], attn_sb[:])
                    dau = fpool.tile([128, 2 * NPAIR], dt, tag="dau")
                    nc.vector.tensor_copy(dau[:], ps_attn[:])
                    nc.sync.dma_start(dbg_au[:], dau[:])

    nc.compile()
    return nc


_NC = None


def _get_nc():
    global _NC
    if _NC is None:
        _NC = build_nc()
    return _NC


def build_in_maps(inputs):
    x = np.ascontiguousarray(np.asarray(inputs["x"], dtype=np.float32)).reshape(B, D)
    cache_k = np.asarray(inputs["cache_k"], dtype=np.float32)
    cache_v = np.asarray(inputs["cache_v"], dtype=np.float32)
    Wq = np.asarray(inputs["Wq"], dtype=np.float32)
    Wk = np.asarray(inputs["Wk"], dtype=np.float32)
    Wv = np.asarray(inputs["Wv"], dtype=np.float32)
    Wo = np.asarray(inputs["Wo"], dtype=np.float32)
    bq = np.asarray(inputs["bq"], dtype=np.float32)
    bk = np.asarray(inputs["bk"], dtype=np.float32)
    bv = np.asarray(inputs["bv"], dtype=np.float32)
    assert int(inputs.get("num_heads", H)) == H

    xT = np.ascontiguousarray(x.T)                       # (D, B)
    eye = np.eye(B, dtype=np.float32)
    ones_p = np.ones((128, 1), np.float32)
    ones_r = np.ones((1, 128), np.float32)
    ones_b = np.ones((1, B), np.float32)

    in_maps = []
    for c in range(NCORES):
        h0 = c * HLOC
        sl = slice(h0 * DK, (h0 + HLOC) * DK)
        # K cache, transposed per pair to (dk, L); pair index = hl*16 + b
        ck = cache_k[:, h0 : h0 + HLOC]                  # (B, HLOC, L, DK)
        kt = np.ascontiguousarray(
            ck.transpose(1, 0, 3, 2).reshape(NPAIR, 128, L)
        )
        # V cache, chunk-partition-major: img[pair, p, j*128+d] = V[j*128+p, d]
        cv = cache_v[:, h0 : h0 + HLOC].reshape(B, HLOC, NCH, 128, DK)
        vi = np.ascontiguousarray(
            cv.transpose(1, 0, 3, 2, 4).reshape(NPAIR, 128, L)
        )
        in_maps.append(
            {
                "kt_img": kt,
                "v_img": vi,
                "wq": np.ascontiguousarray(Wq[:, sl]),
                "wk": np.ascontiguousarray(Wk[:, sl]),
                "wv": np.ascontiguousarray(Wv[:, sl]),
                "wo": np.ascontiguousarray(Wo[sl, :]),
                "bq": np.ascontiguousarray(bq[sl]).reshape(1, -1),
                "bk": np.ascontiguousarray(bk[sl]).reshape(1, -1),
                "bv": np.ascontiguousarray(bv[sl]).reshape(1, -1),
                "xT": xT,
                "eye16": eye,
                "ones_p": ones_p,
                "ones_r": ones_r,
                "ones_b": ones_b,
            }
        )
    return in_maps


def kernel(**inputs):
    cache_k = np.asarray(inputs["cache_k"], dtype=np.float32)
    cache_v = np.asarray(inputs["cache_v"], dtype=np.float32)
    bo = np.asarray(inputs["bo"], dtype=np.float32)

    in_maps = build_in_maps(inputs)
    nc = _get_nc()
    res = bass_utils.run_bass_kernel_spmd(nc, in_maps, core_ids=list(range(NCORES)))

    out = np.zeros((B, D), np.float64)
    k_new = np.empty((B, H, 1, DK), np.float32)
    v_new = np.empty((B, H, 1, DK), np.float32)
    for c in range(NCORES):
        h0 = c * HLOC
        out += res.results[c]["out_p"]
        kT = res.results[c]["k_new"]                     # (128, NPAIR)
        vn = res.results[c]["v_new"]                     # (B, HLOC*DK)
        for hl in range(HLOC):
            k_new[:, h0 + hl, 0, :] = kT[:, hl * B : (hl + 1) * B].T
            v_new[:, h0 + hl, 0, :] = vn[:, hl * DK : (hl + 1) * DK]

    out = (out + bo[None, :]).astype(np.float32).reshape(B, S, D)
    K_full = np.concatenate([cache_k, k_new], axis=2)
    V_full = np.concatenate([cache_v, v_new], axis=2)
    return out, K_full, V_full


# revision 29
# speedup vs baseline: 1.1581x; 1.0065x over previous
"""Trainium2 Bass kernel: cached decoder multi-head self-attention (B=16, S=1,
D=2048, H=16, L=4096), tensor-parallel over heads across 8 NeuronCores.

Per core (2 heads x 16 batches = 32 (b,h) pairs):
  - QKV projections for its head group (column-sharded weights),
  - streaming attention over the 4096-entry KV cache (+1 new token),
  - row-sharded output projection producing a partial (16, 2048) result.
Host packs per-pair SBUF-image cache buffers (K transposed to (dk, L); V
chunk-partition-major) so device DMA is fully contiguous, and sums the 8
partial outputs. Softmax normalization is deferred past the PV matmul
(single reciprocal scale per pair).
"""
import math
import os
import re
import sys

sys.path.insert(0, "/opt/trn_rl_repo")

import numpy as np

import concourse.bass as bass
import concourse.tile as tile
from concourse import bacc, mybir
from concourse import bass_utils

# ---------------------------------------------------------------- problem dims
B, S, D, H, L, DK = 16, 1, 2048, 16, 4096, 128
NCORES = 8
HLOC = H // NCORES            # heads per core = 2
NPAIR = B * HLOC              # (b, h) pairs per core = 32
NCH = L // 128                # 128-key chunks per pair = 32
DCH = D // 128                # 128-row chunks of the model dim = 16
SCALE = 1.0 / math.sqrt(DK)
F32 = mybir.dt.float32
F32R = mybir.dt.float32r
USE_F32R = True               # reduced-precision PE mode for the cache matmuls


def build_nc():
    nc = bacc.Bacc("TRN2", target_bir_lowering=False, debug=False, num_devices=1)

    dt = F32
    dtr = F32R if USE_F32R else F32
    kt_img = nc.dram_tensor("kt_img", [NPAIR, 128, L], dtr, kind="ExternalInput").ap()
    v_img = nc.dram_tensor("v_img", [NPAIR, 128, L], dt, kind="ExternalInput").ap()
    wq_d = nc.dram_tensor("wq", [D, HLOC * DK], dt, kind="ExternalInput").ap()
    wk_d = nc.dram_tensor("wk", [D, HLOC * DK], dt, kind="ExternalInput").ap()
    wv_d = nc.dram_tensor("wv", [D, HLOC * DK], dt, kind="ExternalInput").ap()
    wo_d = nc.dram_tensor("wo", [HLOC * DK, D], dt, kind="ExternalInput").ap()
    bq_d = nc.dram_tensor("bq", [1, HLOC * DK], dt, kind="ExternalInput").ap()
    bk_d = nc.dram_tensor("bk", [1, HLOC * DK], dt, kind="ExternalInput").ap()
    bv_d = nc.dram_tensor("bv", [1, HLOC * DK], dt, kind="ExternalInput").ap()
    xT_d = nc.dram_tensor("xT", [D, B], dt, kind="ExternalInput").ap()
    eye_d = nc.dram_tensor("eye16", [B, B], dt, kind="ExternalInput").ap()
    onp_d = nc.dram_tensor("ones_p", [128, 1], dt, kind="ExternalInput").ap()
    onr_d = nc.dram_tensor("ones_r", [1, 128], dt, kind="ExternalInput").ap()
    onb_d = nc.dram_tensor("ones_b", [1, B], dt, kind="ExternalInput").ap()

    out_d = nc.dram_tensor("out_p", [B, D], dt, kind="ExternalOutput").ap()
    kn_d = nc.dram_tensor("k_new", [128, NPAIR], dt, kind="ExternalOutput").ap()
    vn_d = nc.dram_tensor("v_new", [B, HLOC * DK], dt, kind="ExternalOutput").ap()
    DBG = os.environ.get("KDBG", "0") == "1"
    if DBG:
        dbg_q = nc.dram_tensor("dbg_q", [128, 2 * NPAIR], dt, kind="ExternalOutput").ap()
        dbg_e = nc.dram_tensor("dbg_e", [B, HLOC], dt, kind="ExternalOutput").ap()
        dbg_d = nc.dram_tensor("dbg_d", [1, NPAIR], dt, kind="ExternalOutput").ap()
        dbg_r = nc.dram_tensor("dbg_r", [128, NPAIR], dt, kind="ExternalOutput").ap()
        dbg_a = nc.dram_tensor("dbg_a", [128, NPAIR], dt, kind="ExternalOutput").ap()
        dbg_au = nc.dram_tensor("dbg_au", [128, 2 * NPAIR], dt, kind="ExternalOutput").ap()

    Exp = mybir.ActivationFunctionType.Exp

    with tile.TileContext(nc) as tc:
        with (
            tc.tile_pool(name="const", bufs=1) as cpool,
            tc.tile_pool(name="wts", bufs=1) as wpool,
            tc.tile_pool(name="proj", bufs=1) as ppool,
            tc.tile_pool(name="kt", bufs=3) as ktpool,
            tc.tile_pool(name="vv", bufs=3) as vvpool,
            tc.tile_pool(name="probs", bufs=4) as prpool,
            tc.tile_pool(name="fin", bufs=1) as fpool,
            tc.tile_pool(name="psA", bufs=1, space="PSUM") as psA,
            tc.tile_pool(name="psD", bufs=1, space="PSUM") as psD,
        ):
            # ---------------- constants / weights / activations to SBUF
            eye_sb = cpool.tile([B, B], dt)
            nc.scalar.dma_start(eye_sb[:], eye_d[:])
            onp_sb = cpool.tile([128, 1], dt)
            nc.scalar.dma_start(onp_sb[:], onp_d[:])
            onr_sb = cpool.tile([1, 128], dt)
            nc.scalar.dma_start(onr_sb[:], onr_d[:])
            onb_sb = cpool.tile([1, B], dt)
            nc.scalar.dma_start(onb_sb[:], onb_d[:])
            bq_sb = cpool.tile([1, HLOC * DK], dt)
            nc.scalar.dma_start(bq_sb[:], bq_d[:])
            bk_sb = cpool.tile([1, HLOC * DK], dt)
            nc.scalar.dma_start(bk_sb[:], bk_d[:])
            bv_sb = cpool.tile([1, HLOC * DK], dt)
            nc.scalar.dma_start(bv_sb[:], bv_d[:])
            xT_sb = cpool.tile([128, DCH * B], dt)
            nc.scalar.dma_start(
                xT_sb[:].rearrange("p (c b) -> p c b", c=DCH),
                xT_d.rearrange("(c p) b -> p c b", p=128),
            )
            wq_sb = wpool.tile([128, DCH * HLOC * DK], dt)
            nc.scalar.dma_start(
                wq_sb[:].rearrange("p (c n) -> p c n", c=DCH),
                wq_d.rearrange("(c p) n -> p c n", p=128),
            )
            wk_sb = wpool.tile([128, DCH * HLOC * DK], dt)
            nc.scalar.dma_start(
                wk_sb[:].rearrange("p (c n) -> p c n", c=DCH),
                wk_d.rearrange("(c p) n -> p c n", p=128),
            )
            wv_sb = wpool.tile([128, DCH * HLOC * DK], dt)
            nc.scalar.dma_start(
                wv_sb[:].rearrange("p (c n) -> p c n", c=DCH),
                wv_d.rearrange("(c p) n -> p c n", p=128),
            )
            wo_sb = wpool.tile([128, HLOC * D], dt)
            nc.scalar.dma_start(
                wo_sb[:].rearrange("p (c n) -> p c n", c=HLOC),
                wo_d.rearrange("(c p) n -> p c n", p=128),
            )

            HD = HLOC * DK  # 256

            # ---------------- projections
            # qT2: each pair's q duplicated into two adjacent columns (fp32r
            # matmuls need even moving/dst free counts). col = 2*pair + {0,1}
            qT2_sb = ppool.tile([128, 2 * NPAIR], dtr)
            kT_sb = ppool.tile([128, NPAIR], dt)
            v_sb = ppool.tile([B, HD], dt)         # (batch, hl*dk)
            e_sb = ppool.tile([B, HLOC], dt)       # exp(new-key scores)

            with tc.tile_pool(name="psP", bufs=1, space="PSUM") as psP:
                for wsb, bsb, which in ((wq_sb, bq_sb, "q"), (wk_sb, bk_sb, "k")):
                    for hl in range(HLOC):
                        ps_qt = psP.tile([128, B], dt, tag="qt")
                        for c in range(DCH):
                            nc.tensor.matmul(
                                ps_qt[:],
                                wsb[:, c * HD + hl * DK : c * HD + (hl + 1) * DK],
                                xT_sb[:, c * B : (c + 1) * B],
                                start=(c == 0),
                                stop=False,
                            )
                        nc.tensor.matmul(
                            ps_qt[:],
                            bsb[0:1, hl * DK : (hl + 1) * DK],
                            onb_sb[:],
                            start=False,
                            stop=True,
                        )
                        if which == "q":
                            qslice = qT2_sb[
                                :, 2 * hl * B : 2 * (hl + 1) * B
                            ].rearrange("p (c t) -> p c t", t=2)
                            nc.vector.tensor_copy(
                                qslice[:, :, 0:1],
                                ps_qt[:].rearrange("p (c t) -> p c t", t=1),
                            )
                            nc.vector.tensor_copy(
                                qslice[:, :, 1:2],
                                ps_qt[:].rearrange("p (c t) -> p c t", t=1),
                            )
                        else:
                            nc.vector.tensor_copy(
                                kT_sb[:, hl * B : (hl + 1) * B], ps_qt[:]
                            )
                # V projection, batch-major
                ps_v = psP.tile([B, HD], dt, tag="v")
                for c in range(DCH):
                    nc.tensor.matmul(
                        ps_v[:],
                        xT_sb[:, c * B : (c + 1) * B],
                        wv_sb[:, c * HD : (c + 1) * HD],
                        start=(c == 0),
                        stop=False,
                    )
                nc.tensor.matmul(ps_v[:], onb_sb[:], bv_sb[:], start=False, stop=True)
                nc.vector.tensor_copy(v_sb[:], ps_v[:])

                # new-key scores s_new[b, hl] = q . k_new
                ps_sn = psP.tile([B, HLOC], dt, tag="sn")
                for hl in range(HLOC):
                    tmp = ppool.tile([128, B], dt, tag="tmp")
                    qf32 = (
                        qT2_sb[:, 2 * hl * B : 2 * (hl + 1) * B]
                        .bitcast(F32)
                        .rearrange("p (c t) -> p c t", t=2)
                    )
                    nc.vector.tensor_mul(
                        tmp[:].rearrange("p (c t) -> p c t", t=1),
                        qf32[:, :, 0:1],
                        kT_sb[:, hl * B : (hl + 1) * B].rearrange(
                            "p (c t) -> p c t", t=1
                        ),
                    )
                    nc.tensor.matmul(
                        ps_sn[:, hl : hl + 1], tmp[:], onp_sb[:],
                        start=True, stop=True,
                    )
                nc.scalar.activation(e_sb[:], ps_sn[:], Exp, scale=SCALE)

            # ---------------- pair loop: stream the KV cache
            # fp32r duplicated-column layout: scores/attn psums hold each
            # value twice (cols 2i, 2i+1); part sums come out doubled and the
            # 2.0-valued broadcast constant cancels the factor at normalize.
            ps_attn = psA.tile([128, 2 * NPAIR], dt)
            ps_d = psD.tile([1, NPAIR], dt)        # doubled denominators

            with tc.tile_pool(name="psS", bufs=4, space="PSUM") as psS:
                BF16 = mybir.dt.bfloat16
                for p in range(NPAIR):
                    kt_t = ktpool.tile([128, L], dtr, tag="kt")
                    nc.sync.dma_start(kt_t[:], kt_img[p, :, :])
                    # V cast to bf16 in the DMA (SWDGE): halves SBUF traffic
                    # and enables fast-weight-load bf16 PV matmuls
                    v_t = vvpool.tile([128, L], BF16, tag="vt")
                    nc.gpsimd.dma_start(v_t[:], v_img[p, :, :])

                    ps_s = psS.tile([128, 2 * NCH], dt, tag="s")
                    for j in range(NCH):
                        nc.tensor.matmul(
                            ps_s[:, 2 * j : 2 * j + 2],
                            kt_t[:, j * 128 : (j + 1) * 128],
                            qT2_sb[:, 2 * p : 2 * p + 2],
                            start=True,
                            stop=True,
                        )
                    probs_t = prpool.tile([128, 2 * NCH], BF16, tag="probs")
                    part_t = prpool.tile([128, 1], dt, tag="part")
                    nc.scalar.activation(
                        probs_t[:], ps_s[:], Exp, scale=SCALE, accum_out=part_t[:]
                    )
                    # doubled denominator partial (sums both duplicate cols)
                    nc.tensor.matmul(
                        ps_d[0:1, p : p + 1], part_t[:], onp_sb[:],
                        start=True, stop=True,
                    )
                    for j in range(NCH):
                        nc.tensor.matmul(
                            ps_attn[:, 2 * p : 2 * p + 2],
                            v_t[:, j * 128 : (j + 1) * 128],
                            probs_t[:, 2 * j : 2 * j + 2],
                            start=(j == 0),
                            stop=(j == NCH - 1),
                        )

            # ---------------- new-token contributions + normalization
            with (
                tc.tile_pool(name="psF", bufs=1, space="PSUM") as psF,
                tc.tile_pool(name="psO", bufs=2, space="PSUM") as psO,
            ):
                # new-key contributions in their own (locally-closed) psums
                ps_nk = psF.tile([128, NPAIR], dt, tag="nk")
                ps_de = psF.tile([1, NPAIR], dt, tag="de")
                for hl in range(HLOC):
                    diag_t = fpool.tile([B, B], dt, tag="diag")
                    nc.vector.tensor_scalar_mul(
                        diag_t[:], eye_sb[:], e_sb[:, hl : hl + 1]
                    )
                    nc.tensor.matmul(
                        ps_nk[:, hl * B : (hl + 1) * B],
                        v_sb[:, hl * DK : (hl + 1) * DK],
                        diag_t[:],
                        start=True,
                        stop=True,
                    )
                    nc.tensor.matmul(
                        ps_de[0:1, hl * B : (hl + 1) * B],
                        e_sb[:, hl : hl + 1],
                        eye_sb[:],
                        start=True,
                        stop=True,
                    )
                # d_total = 0.5 * (doubled cached sums) + e_new
                d_sb = fpool.tile([1, NPAIR], dt)
                nc.vector.tensor_copy(d_sb[:], ps_d[:])
                dt_sb = fpool.tile([1, NPAIR], dt)
                nc.vector.scalar_tensor_tensor(
                    dt_sb[:], d_sb[:], 0.5, ps_de[:],
                    op0=mybir.AluOpType.mult, op1=mybir.AluOpType.add,
                )
                r_sb = fpool.tile([1, NPAIR], dt)
                nc.vector.reciprocal(r_sb[:], dt_sb[:])
                ps_r = psF.tile([128, NPAIR], dt, tag="r")
                nc.tensor.matmul(ps_r[:], onr_sb[:], r_sb[:], start=True, stop=True)
                rbc_sb = fpool.tile([128, NPAIR], dt)
                nc.vector.tensor_copy(rbc_sb[:], ps_r[:])
                # attn = (cached-attn + new-key-attn) / d
                nk_sb = fpool.tile([128, NPAIR], dt)
                nc.vector.tensor_copy(nk_sb[:], ps_nk[:])
                asum_sb = fpool.tile([128, NPAIR], dt)
                nc.vector.tensor_add(
                    asum_sb[:].rearrange("p (c t) -> p c t", t=1),
                    ps_attn[:].rearrange("p (c t) -> p c t", t=2)[:, :, 0:1],
                    nk_sb[:].rearrange("p (c t) -> p c t", t=1),
                )
                attn_sb = fpool.tile([128, NPAIR], dt)
                nc.vector.tensor_mul(attn_sb[:], asum_sb[:], rbc_sb[:])

                # output projection: out_partial = attn @ Wo_shard
                out_sb = fpool.tile([B, D], dt)
                for n in range(4):
                    ps_o = psO.tile([B, 512], dt, tag="o")
                    for hl in range(HLOC):
                        nc.tensor.matmul(
                            ps_o[:],
                            attn_sb[:, hl * B : (hl + 1) * B],
                            wo_sb[:, hl * D + n * 512 : hl * D + (n + 1) * 512],
                            start=(hl == 0),
                            stop=(hl == HLOC - 1),
                        )
                    nc.vector.tensor_copy(out_sb[:, n * 512 : (n + 1) * 512], ps_o[:])

                nc.sync.dma_start(out_d[:], out_sb[:])
                nc.sync.dma_start(kn_d[:], kT_sb[:])
                nc.sync.dma_start(vn_d[:], v_sb[:])
                if DBG:
                    dq = fpool.tile([128, 2 * NPAIR], dt, tag="dq")
                    nc.vector.tensor_copy(dq[:], qT2_sb[:].bitcast(F32))
                    nc.sync.dma_start(dbg_q[:], dq[:])
                    nc.sync.dma_start(dbg_e[:], e_sb[:])
                    nc.sync.dma_start(dbg_d[:], d_sb[:])
                    nc.sync.dma_start(dbg_r[:], rbc_sb[:])
                    nc.sync.dma_start(dbg_a[:], attn_sb[:])
                    dau = fpool.tile([128, 2 * NPAIR], dt, tag="dau")
                    nc.vector.tensor_copy(dau[:], ps_attn[:])
                    nc.sync.dma_start(dbg_au[:], dau[:])

    nc.compile()
    return nc


_NC = None


def _get_nc():
    global _NC
    if _NC is None:
        _NC = build_nc()
    return _NC


def build_in_maps(inputs):
    x = np.ascontiguousarray(np.asarray(inputs["x"], dtype=np.float32)).reshape(B, D)
    cache_k = np.asarray(inputs["cache_k"], dtype=np.float32)
    cache_v = np.asarray(inputs["cache_v"], dtype=np.float32)
    Wq = np.asarray(inputs["Wq"], dtype=np.float32)
    Wk = np.asarray(inputs["Wk"], dtype=np.float32)
    Wv = np.asarray(inputs["Wv"], dtype=np.float32)
    Wo = np.asarray(inputs["Wo"], dtype=np.float32)
    bq = np.asarray(inputs["bq"], dtype=np.float32)
    bk = np.asarray(inputs["bk"], dtype=np.float32)
    bv = np.asarray(inputs["bv"], dtype=np.float32)
    assert int(inputs.get("num_heads", H)) == H

    xT = np.ascontiguousarray(x.T)                       # (D, B)
    eye = np.eye(B, dtype=np.float32)
    ones_p = np.ones((128, 1), np.float32)
    ones_r = np.ones((1, 128), np.float32)
    ones_b = np.ones((1, B), np.float32)

    in_maps = []
    for c in range(NCORES):
        h0 = c * HLOC
        sl = slice(h0 * DK, (h0 + HLOC) * DK)
        # K cache, transposed per pair to (dk, L); pair index = hl*16 + b
        ck = cache_k[:, h0 : h0 + HLOC]                  # (B, HLOC, L, DK)
        kt = np.ascontiguousarray(
            ck.transpose(1, 0, 3, 2).reshape(NPAIR, 128, L)
        )
        # V cache, chunk-partition-major: img[pair, p, j*128+d] = V[j*128+p, d]
        cv = cache_v[:, h0 : h0 + HLOC].reshape(B, HLOC, NCH, 128, DK)
        vi = np.ascontiguousarray(
            cv.transpose(1, 0, 3, 2, 4).reshape(NPAIR, 128, L)
        )
        in_maps.append(
            {
                "kt_img": kt,
                "v_img": vi,
                "wq": np.ascontiguousarray(Wq[:, sl]),
                "wk": np.ascontiguousarray(Wk[:, sl]),
                "wv": np.ascontiguousarray(Wv[:, sl]),
                "wo": np.ascontiguousarray(Wo[sl, :]),
                "bq": np.ascontiguousarray(bq[sl]).reshape(1, -1),
                "bk": np.ascontiguousarray(bk[sl]).reshape(1, -1),
                "bv": np.ascontiguousarray(bv[sl]).reshape(1, -1),
                "xT": xT,
                "eye16": eye,
                "ones_p": ones_p,
                "ones_r": ones_r,
                "ones_b": ones_b,
            }
        )
    return in_maps


def kernel(**inputs):
    cache_k = np.asarray(inputs["cache_k"], dtype=np.float32)
    cache_v = np.asarray(inputs["cache_v"], dtype=np.float32)
    bo = np.asarray(inputs["bo"], dtype=np.float32)

    in_maps = build_in_maps(inputs)
    nc = _get_nc()
    res = bass_utils.run_bass_kernel_spmd(nc, in_maps, core_ids=list(range(NCORES)))

    out = np.zeros((B, D), np.float64)
    k_new = np.empty((B, H, 1, DK), np.float32)
    v_new = np.empty((B, H, 1, DK), np.float32)
    for c in range(NCORES):
        h0 = c * HLOC
        out += res.results[c]["out_p"]
        kT = res.results[c]["k_new"]                     # (128, NPAIR)
        vn = res.results[c]["v_new"]                     # (B, HLOC*DK)
        for hl in range(HLOC):
            k_new[:, h0 + hl, 0, :] = kT[:, hl * B : (hl + 1) * B].T
            v_new[:, h0 + hl, 0, :] = vn[:, hl * DK : (hl + 1) * DK]

    out = (out + bo[None, :]).astype(np.float32).reshape(B, S, D)
    K_full = np.concatenate([cache_k, k_new], axis=2)
    V_full = np.concatenate([cache_v, v_new], axis=2)
    return out, K_full, V_full
